# revision 42
# baseline (speedup 1.0000x reference)
"""Segment-mean over ragged contiguous segments of H, SPMD across 8 TRN2 NeuronCores.

out[g, :] = mean(H[start_g : start_g + sizes[g], :]), zero vector for empty segments.

Strategy (data-parallel over graphs, no cross-device communication):
  * Host: split graphs into 8 contiguous, row-balanced shards. Rows are split into
    two streams by segment size: segments with >= SMALL_THRESH rows stream as
    fp8e4m3 (quantization error averages out over the segment; measured max-rel
    error ~8e-3 vs the 2e-2 gate), smaller segments stream as fp16 (~4e-4).
  * fp8 stream: 256-row superblocks, 2 rows per partition (row 2p+i on partition p),
    reduced on TensorE with perf_mode=DoubleRow (K=256 per matmul, 0.5 cyc/row).
  * fp16 stream: 128-row blocks, standard matmuls, accumulated into the same PSUM
    bank as the fp8 stream.
  * One-hot matrices A[row, slot] are generated on VectorE in ONE batched
    tensor_tensor(is_equal) per tile per stream using broadcast access patterns
    (j-iota broadcast along blocks, per-row target slot broadcast along j).
  * Tiles group superblocks so every core's segment span stays <= 128 slots; the
    drain multiplies PSUM by per-slot 1/size and stores fp16; the host scatters
    per-(core, tile) slot ranges back to global segments, summing partials of
    segments that straddle tile boundaries.
"""
import numpy as np

P = 128           # partitions
N_CORES = 8
SLOTS = 128       # output slots per tile (one-hot width; <=128 PSUM partitions)
SMALL_THRESH = 12  # segments with fewer rows stream as fp16
DP = 304          # fp8 row length padded so the DoubleRow Ko step is %16
K8MAX = 16        # max superblocks (256 rows) per tile
TAIL = [4, 2, 1]  # trailing tile sizes (superblocks) to shorten the drain tail

_cache = {}


def _plan(sizes):
    sizes = np.asarray(sizes, np.int64)
    G = sizes.shape[0]
    starts = np.zeros(G + 1, np.int64)
    np.cumsum(sizes, out=starts[1:])
    N = int(starts[-1])

    # contiguous graph ranges, balanced by rows
    bounds = [0]
    for c in range(1, N_CORES):
        target = (N * c) // N_CORES
        g = int(np.searchsorted(starts, target, side="left"))
        if g > 0 and (target - starts[g - 1]) < (starts[g] - target):
            g -= 1
        g = int(min(max(g, bounds[-1]), G))
        bounds.append(g)
    bounds.append(G)

    big_seg = sizes >= SMALL_THRESH      # per-segment stream assignment
    inv_sizes = np.zeros(G, np.float32)
    nz = sizes > 0
    inv_sizes[nz] = (1.0 / sizes[nz].astype(np.float64)).astype(np.float32)

    per_core = []
    for c in range(N_CORES):
        g0, g1 = bounds[c], bounds[c + 1]
        seg_of_row = np.repeat(np.arange(g0, g1, dtype=np.int64), sizes[g0:g1])
        rowmask_big = big_seg[seg_of_row]
        big_segs = seg_of_row[rowmask_big]       # seg id per big row
        small_segs = seg_of_row[~rowmask_big]    # seg id per small row
        per_core.append({
            "g0": g0, "g1": g1, "row0": int(starts[g0]),
            "rows": int(starts[g1] - starts[g0]),
            "rowmask_big": rowmask_big,
            "big_segs": big_segs, "small_segs": small_segs,
        })
    B8 = max((pc["big_segs"].shape[0] + 255) // 256 for pc in per_core)
    assert B8 >= sum(TAIL) + 1, "problem too small for the tail split"

    def assign(kt):
        """Walk tiles; per core compute window [w_next, wend], small rows per
        tile, slot metadata. Returns per-core dicts + per-tile small capacity."""
        T = len(kt)
        complete = sum(kt) >= B8
        out = []
        nsmall = np.zeros((N_CORES, T), np.int64)
        for c in range(N_CORES):
            pc = per_core[c]
            nbig = pc["big_segs"].shape[0]
            sm = pc["small_segs"]
            w_next = pc["g0"]
            sptr = 0           # next unassigned small row
            first_seg = np.full(T, -1, np.int64)
            nslots = np.zeros(T, np.int64)
            small_lo = np.zeros(T, np.int64)
            small_hi = np.zeros(T, np.int64)
            done_big = False
            sb0 = 0
            for t in range(T):
                lo, hi = sb0 * 256, min((sb0 + kt[t]) * 256, nbig)
                sb0 += kt[t]
                if hi > lo:
                    wend = int(pc["big_segs"][hi - 1])
                    if hi == nbig and not done_big:
                        wend = pc["g1"] - 1   # swallow trailing smalls
                        done_big = True
                elif not done_big:            # core had zero big rows
                    wend = pc["g1"] - 1
                    done_big = True
                else:
                    first_seg[t] = -1
                    continue
                if wend < w_next:
                    continue
                first_seg[t] = w_next
                nslots[t] = wend - w_next + 1
                s_hi = int(np.searchsorted(sm, wend, side="right"))
                small_lo[t], small_hi[t] = sptr, s_hi
                nsmall[c, t] = s_hi - sptr
                sptr = s_hi
                if hi > lo:
                    # next window starts at the straddling segment (partials
                    # summed on host) or just after a cleanly-finished one
                    w_next = int(pc["big_segs"][hi - 1]) if hi < nbig else pc["g1"]
                    if hi < nbig and pc["big_segs"][hi] > pc["big_segs"][hi - 1]:
                        w_next += 1
                else:
                    w_next = pc["g1"]
            assert not complete or sptr == sm.shape[0], \
                f"core {c}: unassigned small rows"
            out.append({"first_seg": first_seg, "nslots": nslots,
                        "small_lo": small_lo, "small_hi": small_hi})
        return out, nsmall

    def max_span(kt):
        asg, _ = assign(kt)
        return max(int(a["nslots"].max()) for a in asg)

    # greedy: grow each tile while the largest per-core slot span stays <= 128
    kt = []
    used = 0
    while used < B8:
        k = 1
        assert max_span(kt + [1]) <= SLOTS, "single superblock spans > SLOTS segments"
        while used + k < B8 and k < K8MAX and max_span(kt + [k + 1]) <= SLOTS:
            k += 1
        kt.append(k)
        used += k
    # tail split: replace the last tiles with a descending run so the
    # end-of-kernel serial tail (last DMA -> matmul -> drain -> store) is short
    popped = 0
    while kt and popped < sum(TAIL):
        popped += kt.pop()
    rebuilt = []
    remaining = popped
    for s in TAIL[::-1]:
        if remaining <= 0:
            break
        s = min(s, remaining)
        rebuilt.append(s)
        remaining -= s
    while remaining > 0:
        rebuilt.append(min(K8MAX, remaining))
        remaining -= rebuilt[-1]
    kt.extend(rebuilt[::-1])
    assert sum(kt) == B8
    T = len(kt)

    asg, nsmall = assign(kt)
    S = nsmall.max(axis=0)                   # small rows needed per tile
    sblk = (S + P - 1) // P                  # full 128-row blocks
    sbs = np.zeros(T + 1, np.int64)
    np.cumsum(sblk, out=sbs[1:])
    Bs = int(sbs[T])

    for c in range(N_CORES):
        per_core[c].update(asg[c])

    maxns = np.zeros(T, np.int64)
    for t in range(T):
        maxns[t] = max(int(per_core[c]["nslots"][t]) for c in range(N_CORES))
    out_off = np.zeros(T + 1, np.int64)
    np.cumsum(maxns, out=out_off[1:])

    tfb = np.zeros(T, np.int64)
    np.cumsum(np.asarray(kt[:-1], np.int64), out=tfb[1:])

    # Per-superblock slot windows: each superblock's one-hot covers only an
    # aligned window [ws, ws+wd) of the tile's 128-slot space (PE tile
    # positions quantize the PSUM partition offset to {0,32,64,96} for wd=32,
    # {0,64} for wd=64, {0} for wd>=96). The first superblock of every tile
    # stays full-width: its start=True matmul writes the whole PSUM bank,
    # resetting stale accumulation state before the windowed matmuls land.
    ws = np.zeros(B8, np.int64)
    wd = np.full(B8, P, np.int64)
    for t in range(T):
        for j in range(1, kt[t]):
            sb = int(tfb[t]) + j
            lo, hi = P, -1
            for c in range(N_CORES):
                pc = per_core[c]
                fs = int(pc["first_seg"][t])
                if fs < 0:
                    continue
                nbig = pc["big_segs"].shape[0]
                rlo, rhi = sb * 256, min((sb + 1) * 256, nbig)
                if rhi <= rlo:
                    continue
                lo = min(lo, int(pc["big_segs"][rlo]) - fs)
                hi = max(hi, int(pc["big_segs"][rhi - 1]) - fs)
            if hi < 0:
                ws[sb], wd[sb] = 0, 32      # fully padded superblock
                continue
            assert 0 <= lo <= hi < P
            # DoubleRow matmuls may not use PE matmul packing, so the PSUM
            # window must start at partition 0; width = highest slot ceiled
            # to the 32-partition tile-size granularity
            import os
            if os.environ.get("KERNEL_FULL_WIDTH"):
                ws[sb], wd[sb] = 0, P
            else:
                ws[sb], wd[sb] = 0, min(P, 32 * ((hi // 32) + 1))
    # group superblocks within each tile by window width (one batched
    # one-hot gen per group) and permute the col8 metadata accordingly
    groups = []          # per tile: {width: [local j...]}
    grp_of = {}          # sb -> (width, index within its tile's group)
    moff = []            # per tile: {width: meta pair-offset of group start}
    perm = []
    for t in range(T):
        g = {32: [], 64: [], 96: [], 128: []}
        for j in range(kt[t]):
            g[int(wd[int(tfb[t]) + j])].append(j)
        groups.append(g)
        off = int(tfb[t])
        mo = {}
        for width in (32, 64, 96, 128):
            mo[width] = off
            for idx, j in enumerate(g[width]):
                sb = int(tfb[t]) + j
                grp_of[sb] = (width, idx)
                perm.append(sb)
            off += len(g[width])
        moff.append(mo)
    perm = np.asarray(perm, np.int64)

    return {"G": G, "N": N, "B8": B8, "Bs": Bs, "T": T, "kt": kt, "tfb": tfb,
            "sblk": sblk, "sbs": sbs, "maxns": maxns, "out_off": out_off,
            "per_core": per_core, "inv_sizes": inv_sizes,
            "ws": ws, "wd": wd, "groups": groups, "grp_of": grp_of,
            "moff": moff, "perm": perm,
            "sblkmax": int(sblk.max()) if T else 0}


def _build_program(plan, D):
    import concourse.bacc as bacc
    import concourse.mybir as mybir
    from concourse import tile

    B8, Bs, T, kt = plan["B8"], plan["Bs"], plan["T"], plan["kt"]
    tfb, sblk, sbs = plan["tfb"], plan["sblk"], plan["sbs"]
    maxns, out_off = plan["maxns"], plan["out_off"]
    ws, wd = plan["ws"], plan["wd"]
    groups, grp_of, moff = plan["groups"], plan["grp_of"], plan["moff"]
    f32 = mybir.dt.float32
    f16 = mybir.dt.float16
    f8 = mybir.dt.float8e4
    DR = mybir.MatmulPerfMode.DoubleRow

    nc = bacc.Bacc("TRN2", target_bir_lowering=False, debug=False,
                   num_devices=N_CORES)
    # fp8 planar: h8[p, (2*sb+i)*DP + d] = big row 256*sb + 2p + i
    h8 = nc.declare_dram_parameter("h8", [P, B8 * 2 * DP], f8, isOutput=False)
    # fp16 planar: hs[p, blk*D + d] = small row 128*blk + p
    hs = nc.declare_dram_parameter("hs", [P, max(Bs, 1) * D], f16, isOutput=False)
    # meta = [J (P) | col8 (2*B8) | cols (Bs)] fp16; winv separate fp32
    meta = nc.declare_dram_parameter("meta", [P, P + 2 * B8 + max(Bs, 1)], f16,
                                     isOutput=False)
    winv_d = nc.declare_dram_parameter("winv", [P, T], f32, isOutput=False)
    out = nc.declare_dram_parameter("out", [int(out_off[T]), D], f16, isOutput=True)

    k8max = max(kt)
    sblkmax = max(plan["sblkmax"], 1)
    with tile.TileContext(nc) as tc:
        with (
            tc.tile_pool(name="const", bufs=1) as cpool,
            tc.tile_pool(name="h8buf", bufs=8) as h8pool,
            tc.tile_pool(name="hsbuf", bufs=4) as hspool,
            tc.tile_pool(name="a32buf", bufs=4) as a32pool,
            tc.tile_pool(name="a64buf", bufs=4) as a64pool,
            tc.tile_pool(name="a96buf", bufs=4) as a96pool,
            tc.tile_pool(name="a128buf", bufs=4) as a128pool,
            tc.tile_pool(name="asbuf", bufs=3) as aspool,
            tc.tile_pool(name="obuf", bufs=2) as opool,
            tc.tile_pool(name="psum", bufs=8, space="PSUM") as ppool,
        ):
            apools = {32: a32pool, 64: a64pool, 96: a96pool, 128: a128pool}
            m_t = cpool.tile([P, P + 2 * B8 + max(Bs, 1)], f16)
            # split the meta load so tile 0's A-gen starts without waiting
            # for the whole metadata block
            m0 = P + 2 * kt[0]
            nc.scalar.dma_start(m_t[:, 0:m0], meta[:, 0:m0])
            nc.scalar.dma_start(m_t[:, m0:], meta[:, m0:])
            w_t = cpool.tile([P, T], f32)
            nc.scalar.dma_start(w_t[:], winv_d[:])
            j_t = m_t[:, 0:P]

            for t in range(T):
                k8 = kt[t]
                sb0 = int(tfb[t])
                ns_blk = int(sblk[t])
                ss0 = int(sbs[t])

                h8_t = h8pool.tile([P, 2 * k8max, DP], f8, tag="h8buf")
                nc.sync.dma_start(
                    h8_t[:, 0:2 * k8, :],
                    h8[:, sb0 * 2 * DP:(sb0 + k8) * 2 * DP]
                    .rearrange("p (x d) -> p x d", d=DP))
                # one batched one-hot gen per window-width group; col8
                # metadata is host-permuted into group order per tile
                a_t = {}
                for width in (128, 32, 64, 96):
                    ng = len(groups[t][width])
                    if ng == 0:
                        continue
                    a_t[width] = apools[width].tile(
                        [P, 2 * k8max, width], f8, tag=f"a{width}buf",
                        name=f"a{width}_t")
                    mo = moff[t][width]
                    colw = m_t[:, P + 2 * mo:P + 2 * (mo + ng)]
                    nc.vector.tensor_tensor(
                        a_t[width][:, 0:2 * ng, :],
                        j_t[:, 0:width].unsqueeze(1)
                        .broadcast_to([P, 2 * ng, width]),
                        colw.unsqueeze(2).broadcast_to([P, 2 * ng, width]),
                        mybir.AluOpType.is_equal)

                if ns_blk > 0:
                    hs_t = hspool.tile([P, sblkmax, D], f16, tag="hsbuf")
                    nc.scalar.dma_start(
                        hs_t[:, 0:ns_blk, :],
                        hs[:, ss0 * D:(ss0 + ns_blk) * D]
                        .rearrange("p (x d) -> p x d", d=D))
                    as_t = aspool.tile([P, sblkmax, SLOTS], f16, tag="asbuf")
                    cols = m_t[:, P + 2 * B8 + ss0:P + 2 * B8 + ss0 + ns_blk]
                    nc.vector.tensor_tensor(
                        as_t[:, 0:ns_blk, :],
                        j_t[:, 0:SLOTS].unsqueeze(1).broadcast_to([P, ns_blk, SLOTS]),
                        cols.unsqueeze(2).broadcast_to([P, ns_blk, SLOTS]),
                        mybir.AluOpType.is_equal)

                acc = ppool.tile([SLOTS, D], f32)
                n_mm = k8 + ns_blk
                i = 0
                for j in range(k8):
                    sb = sb0 + j
                    width, gi = grp_of[sb]
                    w0 = int(ws[sb])
                    nc.tensor.matmul(
                        acc[w0:w0 + width, :],
                        a_t[width][:, 2 * gi:2 * gi + 2, :],
                        h8_t[:, 2 * j:2 * j + 2, 0:D],
                        start=(i == 0), stop=(i == n_mm - 1),
                        perf_mode=DR, skip_group_check=True)
                    i += 1
                for blk in range(ns_blk):
                    nc.tensor.matmul(
                        acc[:], as_t[:, blk, :], hs_t[:, blk, :],
                        start=(i == 0), stop=(i == n_mm - 1),
                        skip_group_check=True)
                    i += 1

                mn = int(maxns[t])
                o_t = opool.tile([SLOTS, D], f16, tag="obuf")
                # drain on the ACT engine (out = acc * winv, per-partition
                # scale) so the DVE stays free for one-hot generation
                nc.scalar.mul(o_t[0:mn, :], acc[0:mn, :], w_t[0:mn, t:t + 1])
                # stores ride the Sync ring: the h8 prefetch queue is deep
                # enough (8 bufs) that a drain-gated store between prefetches
                # doesn't starve the PE, and it unloads the ACT ring
                nc.sync.dma_start(
                    out[int(out_off[t]):int(out_off[t]) + mn, :], o_t[0:mn, :])
    nc.compile()
    return nc


def kernel(H, sizes):
    import ml_dtypes
    from concourse.bass_utils import run_bass_kernel_spmd

    f8np = ml_dtypes.float8_e4m3
    H = np.ascontiguousarray(np.asarray(H, np.float32))
    sizes_np = np.asarray(sizes, np.int64)
    N, D = H.shape
    G = sizes_np.shape[0]

    key = (sizes_np.tobytes(), D)
    if key not in _cache:
        plan = _plan(sizes_np)
        assert plan["N"] == N, f"sizes sum {plan['N']} != H rows {N}"
        nc = _build_program(plan, D)
        _cache.clear()
        _cache[key] = (plan, nc)
    plan, nc = _cache[key]

    B8, Bs, T = plan["B8"], plan["Bs"], plan["T"]
    sblk, sbs = plan["sblk"], plan["sbs"]
    inv_sizes = plan["inv_sizes"]
    jmat = np.broadcast_to(np.arange(P, dtype=np.float16), (P, P))
    in_maps = []
    for c in range(N_CORES):
        pc = plan["per_core"][c]
        rows0 = pc["row0"]
        Hc = H[rows0:rows0 + pc["rows"]]
        Hbig = Hc[pc["rowmask_big"]]
        Hsmall = Hc[~pc["rowmask_big"]]
        nbig = Hbig.shape[0]

        h8pad = np.zeros((B8 * 256, DP), f8np)
        h8pad[:nbig, :D] = Hbig.astype(f8np)
        h8planar = np.ascontiguousarray(
            h8pad.reshape(B8, P, 2, DP).transpose(1, 0, 2, 3)
            .reshape(P, B8 * 2 * DP))

        # per-row slot targets, local to each superblock's window;
        # -1 on padding rows -> all-zero one-hot column
        col8 = np.full(B8 * 256, -1.0, np.float32)
        first_seg, nslots = pc["first_seg"], pc["nslots"]
        kt, tfb = plan["kt"], plan["tfb"]
        for t in range(T):
            lo = int(tfb[t]) * 256
            hi = min((int(tfb[t]) + kt[t]) * 256, nbig)
            if hi > lo and first_seg[t] >= 0:
                sbrow = np.arange(lo, hi) // 256
                loc = (pc["big_segs"][lo:hi] - first_seg[t]
                       - plan["ws"][sbrow]).astype(np.float32)
                assert loc.min() >= 0 and \
                    (loc < plan["wd"][sbrow]).all(), "row outside its window"
                col8[lo:hi] = loc
        col8p = np.ascontiguousarray(
            col8.reshape(B8, P, 2).transpose(1, 0, 2)[:, plan["perm"], :]
            .reshape(P, 2 * B8).astype(np.float16))

        hspad = np.zeros((max(Bs, 1) * P, D), np.float16)
        colsf = np.full(max(Bs, 1) * P, -1.0, np.float32)
        for t in range(T):
            slo, shi = int(pc["small_lo"][t]), int(pc["small_hi"][t])
            n = shi - slo
            if n > 0:
                base = int(sbs[t]) * P
                hspad[base:base + n] = Hsmall[slo:shi].astype(np.float16)
                colsf[base:base + n] = (pc["small_segs"][slo:shi]
                                        - first_seg[t]).astype(np.float32)
        hsplanar = np.ascontiguousarray(
            hspad.reshape(max(Bs, 1), P, D).transpose(1, 0, 2)
            .reshape(P, max(Bs, 1) * D))
        colsp = np.ascontiguousarray(
            colsf.reshape(max(Bs, 1), P).T.astype(np.float16))

        winv = np.zeros((P, T), np.float32)
        for t in range(T):
            ns = int(nslots[t])
            if ns > 0:
                fs = int(first_seg[t])
                winv[:ns, t] = inv_sizes[fs:fs + ns]

        meta = np.concatenate([jmat, col8p, colsp], axis=1)
        in_maps.append({"h8": h8planar, "hs": hsplanar,
                        "meta": np.ascontiguousarray(meta),
                        "winv": np.ascontiguousarray(winv)})

    import os, sys
    trace = bool(os.environ.get("KERNEL_TRACE")) and "antenv.axon_hooks" in sys.modules
    kw = {}
    if trace:
        kw = {"trace": True, "tmpdir": os.environ.get("KERNEL_TRACE_DIR") or None}
    res = run_bass_kernel_spmd(nc, in_maps, core_ids=list(range(N_CORES)), **kw)

    global LAST_EXEC_NS
    LAST_EXEC_NS = getattr(res, "exec_time_ns", None)

    out_off = plan["out_off"]
    out_full = np.zeros((G, D), np.float32)
    for c in range(N_CORES):
        pc = plan["per_core"][c]
        dev = np.asarray(res.results[c]["out"], np.float32)
        for t in range(T):
            ns = int(pc["nslots"][t])
            if ns > 0:
                fs = int(pc["first_seg"][t])
                oo = int(out_off[t])
                out_full[fs:fs + ns] += dev[oo:oo + ns]
    return out_full


LAST_EXEC_NS = None


# revision 43
# speedup vs baseline: 1.2345x; 1.2345x over previous
"""Segment-mean over ragged contiguous segments of H, SPMD across 8 TRN2 NeuronCores.

out[g, :] = mean(H[start_g : start_g + sizes[g], :]), zero vector for empty segments.

Strategy (data-parallel over graphs, no cross-device communication):
  * Host: split graphs into 8 contiguous, row-balanced shards. Rows are split into
    two streams by segment size: segments with >= SMALL_THRESH rows stream as
    fp8e4m3 (quantization error averages out over the segment; measured max-rel
    error ~8e-3 vs the 2e-2 gate), smaller segments stream as fp16 (~4e-4).
  * fp8 stream: 256-row superblocks, 2 rows per partition (row 2p+i on partition p),
    reduced on TensorE with perf_mode=DoubleRow (K=256 per matmul, 0.5 cyc/row).
  * fp16 stream: 128-row blocks, standard matmuls, accumulated into the same PSUM
    bank as the fp8 stream.
  * One-hot matrices A[row, slot] are generated on VectorE in ONE batched
    tensor_tensor(is_equal) per tile per stream using broadcast access patterns
    (j-iota broadcast along blocks, per-row target slot broadcast along j).
  * Tiles group superblocks so every core's segment span stays <= 128 slots; the
    drain multiplies PSUM by per-slot 1/size and stores fp16; the host scatters
    per-(core, tile) slot ranges back to global segments, summing partials of
    segments that straddle tile boundaries.
"""
import numpy as np

P = 128           # partitions
N_CORES = 8
SLOTS = 128       # output slots per tile (one-hot width; <=128 PSUM partitions)
SMALL_THRESH = 12  # segments with fewer rows stream as fp16
DP = 304          # fp8 row length padded so the DoubleRow Ko step is %16
K8MAX = 16        # max superblocks (256 rows) per tile
TAIL = [4, 2, 1]  # trailing tile sizes (superblocks) to shorten the drain tail

_cache = {}


def _plan(sizes):
    sizes = np.asarray(sizes, np.int64)
    G = sizes.shape[0]
    starts = np.zeros(G + 1, np.int64)
    np.cumsum(sizes, out=starts[1:])
    N = int(starts[-1])

    # contiguous graph ranges, balanced by rows
    bounds = [0]
    for c in range(1, N_CORES):
        target = (N * c) // N_CORES
        g = int(np.searchsorted(starts, target, side="left"))
        if g > 0 and (target - starts[g - 1]) < (starts[g] - target):
            g -= 1
        g = int(min(max(g, bounds[-1]), G))
        bounds.append(g)
    bounds.append(G)

    big_seg = sizes >= SMALL_THRESH      # per-segment stream assignment
    inv_sizes = np.zeros(G, np.float32)
    nz = sizes > 0
    inv_sizes[nz] = (1.0 / sizes[nz].astype(np.float64)).astype(np.float32)

    per_core = []
    for c in range(N_CORES):
        g0, g1 = bounds[c], bounds[c + 1]
        seg_of_row = np.repeat(np.arange(g0, g1, dtype=np.int64), sizes[g0:g1])
        rowmask_big = big_seg[seg_of_row]
        big_segs = seg_of_row[rowmask_big]       # seg id per big row
        small_segs = seg_of_row[~rowmask_big]    # seg id per small row
        per_core.append({
            "g0": g0, "g1": g1, "row0": int(starts[g0]),
            "rows": int(starts[g1] - starts[g0]),
            "rowmask_big": rowmask_big,
            "big_segs": big_segs, "small_segs": small_segs,
        })
    B8 = max((pc["big_segs"].shape[0] + 255) // 256 for pc in per_core)
    assert B8 >= sum(TAIL) + 1, "problem too small for the tail split"

    def assign(kt):
        """Walk tiles; per core compute window [w_next, wend], small rows per
        tile, slot metadata. Returns per-core dicts + per-tile small capacity."""
        T = len(kt)
        complete = sum(kt) >= B8
        out = []
        nsmall = np.zeros((N_CORES, T), np.int64)
        for c in range(N_CORES):
            pc = per_core[c]
            nbig = pc["big_segs"].shape[0]
            sm = pc["small_segs"]
            w_next = pc["g0"]
            sptr = 0           # next unassigned small row
            first_seg = np.full(T, -1, np.int64)
            nslots = np.zeros(T, np.int64)
            small_lo = np.zeros(T, np.int64)
            small_hi = np.zeros(T, np.int64)
            done_big = False
            sb0 = 0
            for t in range(T):
                lo, hi = sb0 * 256, min((sb0 + kt[t]) * 256, nbig)
                sb0 += kt[t]
                if hi > lo:
                    wend = int(pc["big_segs"][hi - 1])
                    if hi == nbig and not done_big:
                        wend = pc["g1"] - 1   # swallow trailing smalls
                        done_big = True
                elif not done_big:            # core had zero big rows
                    wend = pc["g1"] - 1
                    done_big = True
                else:
                    first_seg[t] = -1
                    continue
                if wend < w_next:
                    continue
                first_seg[t] = w_next
                nslots[t] = wend - w_next + 1
                s_hi = int(np.searchsorted(sm, wend, side="right"))
                small_lo[t], small_hi[t] = sptr, s_hi
                nsmall[c, t] = s_hi - sptr
                sptr = s_hi
                if hi > lo:
                    # next window starts at the straddling segment (partials
                    # summed on host) or just after a cleanly-finished one
                    w_next = int(pc["big_segs"][hi - 1]) if hi < nbig else pc["g1"]
                    if hi < nbig and pc["big_segs"][hi] > pc["big_segs"][hi - 1]:
                        w_next += 1
                else:
                    w_next = pc["g1"]
            assert not complete or sptr == sm.shape[0], \
                f"core {c}: unassigned small rows"
            out.append({"first_seg": first_seg, "nslots": nslots,
                        "small_lo": small_lo, "small_hi": small_hi})
        return out, nsmall

    def max_span(kt):
        asg, _ = assign(kt)
        return max(int(a["nslots"].max()) for a in asg)

    # greedy: grow each tile while the largest per-core slot span stays <= 128
    kt = []
    used = 0
    while used < B8:
        k = 1
        assert max_span(kt + [1]) <= SLOTS, "single superblock spans > SLOTS segments"
        while used + k < B8 and k < K8MAX and max_span(kt + [k + 1]) <= SLOTS:
            k += 1
        kt.append(k)
        used += k
    # tail split: replace the last tiles with a descending run so the
    # end-of-kernel serial tail (last DMA -> matmul -> drain -> store) is short
    popped = 0
    while kt and popped < sum(TAIL):
        popped += kt.pop()
    rebuilt = []
    remaining = popped
    for s in TAIL[::-1]:
        if remaining <= 0:
            break
        s = min(s, remaining)
        rebuilt.append(s)
        remaining -= s
    while remaining > 0:
        rebuilt.append(min(K8MAX, remaining))
        remaining -= rebuilt[-1]
    kt.extend(rebuilt[::-1])
    assert sum(kt) == B8
    T = len(kt)

    asg, nsmall = assign(kt)
    S = nsmall.max(axis=0)                   # small rows needed per tile
    sblk = (S + P - 1) // P                  # full 128-row blocks
    sbs = np.zeros(T + 1, np.int64)
    np.cumsum(sblk, out=sbs[1:])
    Bs = int(sbs[T])

    for c in range(N_CORES):
        per_core[c].update(asg[c])

    maxns = np.zeros(T, np.int64)
    for t in range(T):
        maxns[t] = max(int(per_core[c]["nslots"][t]) for c in range(N_CORES))
    out_off = np.zeros(T + 1, np.int64)
    np.cumsum(maxns, out=out_off[1:])

    tfb = np.zeros(T, np.int64)
    np.cumsum(np.asarray(kt[:-1], np.int64), out=tfb[1:])

    # Per-superblock slot windows: each superblock's one-hot covers only an
    # aligned window [ws, ws+wd) of the tile's 128-slot space (PE tile
    # positions quantize the PSUM partition offset to {0,32,64,96} for wd=32,
    # {0,64} for wd=64, {0} for wd>=96). The first superblock of every tile
    # stays full-width: its start=True matmul writes the whole PSUM bank,
    # resetting stale accumulation state before the windowed matmuls land.
    ws = np.zeros(B8, np.int64)
    wd = np.full(B8, P, np.int64)
    for t in range(T):
        for j in range(1, kt[t]):
            sb = int(tfb[t]) + j
            lo, hi = P, -1
            for c in range(N_CORES):
                pc = per_core[c]
                fs = int(pc["first_seg"][t])
                if fs < 0:
                    continue
                nbig = pc["big_segs"].shape[0]
                rlo, rhi = sb * 256, min((sb + 1) * 256, nbig)
                if rhi <= rlo:
                    continue
                lo = min(lo, int(pc["big_segs"][rlo]) - fs)
                hi = max(hi, int(pc["big_segs"][rhi - 1]) - fs)
            if hi < 0:
                ws[sb], wd[sb] = 0, 32      # fully padded superblock
                continue
            assert 0 <= lo <= hi < P
            # DoubleRow matmuls may not use PE matmul packing, so the PSUM
            # window must start at partition 0; width = highest slot ceiled
            # to the 32-partition tile-size granularity
            import os
            if os.environ.get("KERNEL_FULL_WIDTH"):
                ws[sb], wd[sb] = 0, P
            else:
                ws[sb], wd[sb] = 0, min(P, 32 * ((hi // 32) + 1))
    # group superblocks within each tile by window width (one batched
    # one-hot gen per group) and permute the col8 metadata accordingly
    groups = []          # per tile: {width: [local j...]}
    grp_of = {}          # sb -> (width, index within its tile's group)
    moff = []            # per tile: {width: meta pair-offset of group start}
    perm = []
    for t in range(T):
        g = {32: [], 64: [], 96: [], 128: []}
        for j in range(kt[t]):
            g[int(wd[int(tfb[t]) + j])].append(j)
        groups.append(g)
        off = int(tfb[t])
        mo = {}
        for width in (32, 64, 96, 128):
            mo[width] = off
            for idx, j in enumerate(g[width]):
                sb = int(tfb[t]) + j
                grp_of[sb] = (width, idx)
                perm.append(sb)
            off += len(g[width])
        moff.append(mo)
    perm = np.asarray(perm, np.int64)

    return {"G": G, "N": N, "B8": B8, "Bs": Bs, "T": T, "kt": kt, "tfb": tfb,
            "sblk": sblk, "sbs": sbs, "maxns": maxns, "out_off": out_off,
            "per_core": per_core, "inv_sizes": inv_sizes,
            "ws": ws, "wd": wd, "groups": groups, "grp_of": grp_of,
            "moff": moff, "perm": perm,
            "sblkmax": int(sblk.max()) if T else 0}


def _build_program(plan, D):
    import concourse.bacc as bacc
    import concourse.mybir as mybir
    from concourse import tile

    B8, Bs, T, kt = plan["B8"], plan["Bs"], plan["T"], plan["kt"]
    tfb, sblk, sbs = plan["tfb"], plan["sblk"], plan["sbs"]
    maxns, out_off = plan["maxns"], plan["out_off"]
    ws, wd = plan["ws"], plan["wd"]
    groups, grp_of, moff = plan["groups"], plan["grp_of"], plan["moff"]
    f32 = mybir.dt.float32
    f16 = mybir.dt.float16
    f8 = mybir.dt.float8e4
    DR = mybir.MatmulPerfMode.DoubleRow

    nc = bacc.Bacc("TRN2", target_bir_lowering=False, debug=False,
                   num_devices=N_CORES)
    # fp8 planar: h8[p, (2*sb+i)*DP + d] = big row 256*sb + 2p + i
    h8 = nc.declare_dram_parameter("h8", [P, B8 * 2 * DP], f8, isOutput=False)
    # fp16 planar: hs[p, blk*D + d] = small row 128*blk + p
    hs = nc.declare_dram_parameter("hs", [P, max(Bs, 1) * D], f16, isOutput=False)
    # meta = [J (P) | col8 (2*B8) | cols (Bs)] fp16; winv separate fp32
    meta = nc.declare_dram_parameter("meta", [P, P + 2 * B8 + max(Bs, 1)], f16,
                                     isOutput=False)
    winv_d = nc.declare_dram_parameter("winv", [P, T], f32, isOutput=False)
    out = nc.declare_dram_parameter("out", [int(out_off[T]), D], f16, isOutput=True)

    k8max = max(kt)
    sblkmax = max(plan["sblkmax"], 1)
    with tile.TileContext(nc) as tc:
        with (
            tc.tile_pool(name="const", bufs=1) as cpool,
            tc.tile_pool(name="h8buf", bufs=8) as h8pool,
            tc.tile_pool(name="hsbuf", bufs=4) as hspool,
            tc.tile_pool(name="a32buf", bufs=4) as a32pool,
            tc.tile_pool(name="a64buf", bufs=4) as a64pool,
            tc.tile_pool(name="a96buf", bufs=4) as a96pool,
            tc.tile_pool(name="a128buf", bufs=4) as a128pool,
            tc.tile_pool(name="asbuf", bufs=3) as aspool,
            tc.tile_pool(name="obuf", bufs=2) as opool,
            tc.tile_pool(name="psum", bufs=8, space="PSUM") as ppool,
        ):
            apools = {32: a32pool, 64: a64pool, 96: a96pool, 128: a128pool}
            m_t = cpool.tile([P, P + 2 * B8 + max(Bs, 1)], f16)
            # split the meta load so tile 0's A-gen starts without waiting
            # for the whole metadata block
            m0 = P + 2 * kt[0]
            nc.scalar.dma_start(m_t[:, 0:m0], meta[:, 0:m0])
            nc.scalar.dma_start(m_t[:, m0:], meta[:, m0:])
            w_t = cpool.tile([P, T], f32)
            nc.scalar.dma_start(w_t[:], winv_d[:])
            j_t = m_t[:, 0:P]

            for t in range(T):
                k8 = kt[t]
                sb0 = int(tfb[t])
                ns_blk = int(sblk[t])
                ss0 = int(sbs[t])

                h8_t = h8pool.tile([P, 2 * k8max, DP], f8, tag="h8buf")
                nc.sync.dma_start(
                    h8_t[:, 0:2 * k8, :],
                    h8[:, sb0 * 2 * DP:(sb0 + k8) * 2 * DP]
                    .rearrange("p (x d) -> p x d", d=DP))
                # one batched one-hot gen per window-width group; col8
                # metadata is host-permuted into group order per tile
                a_t = {}
                for width in (128, 32, 64, 96):
                    ng = len(groups[t][width])
                    if ng == 0:
                        continue
                    a_t[width] = apools[width].tile(
                        [P, 2 * k8max, width], f8, tag=f"a{width}buf",
                        name=f"a{width}_t")
                    mo = moff[t][width]
                    colw = m_t[:, P + 2 * mo:P + 2 * (mo + ng)]
                    nc.vector.tensor_tensor(
                        a_t[width][:, 0:2 * ng, :],
                        j_t[:, 0:width].unsqueeze(1)
                        .broadcast_to([P, 2 * ng, width]),
                        colw.unsqueeze(2).broadcast_to([P, 2 * ng, width]),
                        mybir.AluOpType.is_equal)

                if ns_blk > 0:
                    hs_t = hspool.tile([P, sblkmax, D], f16, tag="hsbuf")
                    nc.scalar.dma_start(
                        hs_t[:, 0:ns_blk, :],
                        hs[:, ss0 * D:(ss0 + ns_blk) * D]
                        .rearrange("p (x d) -> p x d", d=D))
                    as_t = aspool.tile([P, sblkmax, SLOTS], f16, tag="asbuf")
                    cols = m_t[:, P + 2 * B8 + ss0:P + 2 * B8 + ss0 + ns_blk]
                    nc.vector.tensor_tensor(
                        as_t[:, 0:ns_blk, :],
                        j_t[:, 0:SLOTS].unsqueeze(1).broadcast_to([P, ns_blk, SLOTS]),
                        cols.unsqueeze(2).broadcast_to([P, ns_blk, SLOTS]),
                        mybir.AluOpType.is_equal)

                acc = ppool.tile([SLOTS, D], f32)
                n_mm = k8 + ns_blk
                i = 0
                for j in range(k8):
                    sb = sb0 + j
                    width, gi = grp_of[sb]
                    w0 = int(ws[sb])
                    nc.tensor.matmul(
                        acc[w0:w0 + width, :],
                        a_t[width][:, 2 * gi:2 * gi + 2, :],
                        h8_t[:, 2 * j:2 * j + 2, 0:D],
                        start=(i == 0), stop=(i == n_mm - 1),
                        perf_mode=DR, skip_group_check=True)
                    i += 1
                for blk in range(ns_blk):
                    nc.tensor.matmul(
                        acc[:], as_t[:, blk, :], hs_t[:, blk, :],
                        start=(i == 0), stop=(i == n_mm - 1),
                        skip_group_check=True)
                    i += 1

                mn = int(maxns[t])
                o_t = opool.tile([SLOTS, D], f16, tag="obuf")
                # drain on the ACT engine (out = acc * winv, per-partition
                # scale) so the DVE stays free for one-hot generation
                nc.scalar.mul(o_t[0:mn, :], acc[0:mn, :], w_t[0:mn, t:t + 1])
                # stores on the ACT HWDGE ring: keeps the Sync ring a pure
                # H-prefetch FIFO (a drain-gated store would head-of-line
                # block the next H load there)
                nc.scalar.dma_start(
                    out[int(out_off[t]):int(out_off[t]) + mn, :], o_t[0:mn, :])
    nc.compile()
    return nc


def kernel(H, sizes):
    import ml_dtypes
    from concourse.bass_utils import run_bass_kernel_spmd

    f8np = ml_dtypes.float8_e4m3
    H = np.ascontiguousarray(np.asarray(H, np.float32))
    sizes_np = np.asarray(sizes, np.int64)
    N, D = H.shape
    G = sizes_np.shape[0]

    key = (sizes_np.tobytes(), D)
    if key not in _cache:
        plan = _plan(sizes_np)
        assert plan["N"] == N, f"sizes sum {plan['N']} != H rows {N}"
        nc = _build_program(plan, D)
        _cache.clear()
        _cache[key] = (plan, nc)
    plan, nc = _cache[key]

    B8, Bs, T = plan["B8"], plan["Bs"], plan["T"]
    sblk, sbs = plan["sblk"], plan["sbs"]
    inv_sizes = plan["inv_sizes"]
    jmat = np.broadcast_to(np.arange(P, dtype=np.float16), (P, P))
    in_maps = []
    for c in range(N_CORES):
        pc = plan["per_core"][c]
        rows0 = pc["row0"]
        Hc = H[rows0:rows0 + pc["rows"]]
        Hbig = Hc[pc["rowmask_big"]]
        Hsmall = Hc[~pc["rowmask_big"]]
        nbig = Hbig.shape[0]

        h8pad = np.zeros((B8 * 256, DP), f8np)
        h8pad[:nbig, :D] = Hbig.astype(f8np)
        h8planar = np.ascontiguousarray(
            h8pad.reshape(B8, P, 2, DP).transpose(1, 0, 2, 3)
            .reshape(P, B8 * 2 * DP))

        # per-row slot targets, local to each superblock's window;
        # -1 on padding rows -> all-zero one-hot column
        col8 = np.full(B8 * 256, -1.0, np.float32)
        first_seg, nslots = pc["first_seg"], pc["nslots"]
        kt, tfb = plan["kt"], plan["tfb"]
        for t in range(T):
            lo = int(tfb[t]) * 256
            hi = min((int(tfb[t]) + kt[t]) * 256, nbig)
            if hi > lo and first_seg[t] >= 0:
                sbrow = np.arange(lo, hi) // 256
                loc = (pc["big_segs"][lo:hi] - first_seg[t]
                       - plan["ws"][sbrow]).astype(np.float32)
                assert loc.min() >= 0 and \
                    (loc < plan["wd"][sbrow]).all(), "row outside its window"
                col8[lo:hi] = loc
        col8p = np.ascontiguousarray(
            col8.reshape(B8, P, 2).transpose(1, 0, 2)[:, plan["perm"], :]
            .reshape(P, 2 * B8).astype(np.float16))

        hspad = np.zeros((max(Bs, 1) * P, D), np.float16)
        colsf = np.full(max(Bs, 1) * P, -1.0, np.float32)
        for t in range(T):
            slo, shi = int(pc["small_lo"][t]), int(pc["small_hi"][t])
            n = shi - slo
            if n > 0:
                base = int(sbs[t]) * P
                hspad[base:base + n] = Hsmall[slo:shi].astype(np.float16)
                colsf[base:base + n] = (pc["small_segs"][slo:shi]
                                        - first_seg[t]).astype(np.float32)
        hsplanar = np.ascontiguousarray(
            hspad.reshape(max(Bs, 1), P, D).transpose(1, 0, 2)
            .reshape(P, max(Bs, 1) * D))
        colsp = np.ascontiguousarray(
            colsf.reshape(max(Bs, 1), P).T.astype(np.float16))

        winv = np.zeros((P, T), np.float32)
        for t in range(T):
            ns = int(nslots[t])
            if ns > 0:
                fs = int(first_seg[t])
                winv[:ns, t] = inv_sizes[fs:fs + ns]

        meta = np.concatenate([jmat, col8p, colsp], axis=1)
        in_maps.append({"h8": h8planar, "hs": hsplanar,
                        "meta": np.ascontiguousarray(meta),
                        "winv": np.ascontiguousarray(winv)})

    import os, sys
    trace = bool(os.environ.get("KERNEL_TRACE")) and "antenv.axon_hooks" in sys.modules
    kw = {}
    if trace:
        kw = {"trace": True, "tmpdir": os.environ.get("KERNEL_TRACE_DIR") or None}
    res = run_bass_kernel_spmd(nc, in_maps, core_ids=list(range(N_CORES)), **kw)

    global LAST_EXEC_NS
    LAST_EXEC_NS = getattr(res, "exec_time_ns", None)

    out_off = plan["out_off"]
    out_full = np.zeros((G, D), np.float32)
    for c in range(N_CORES):
        pc = plan["per_core"][c]
        dev = np.asarray(res.results[c]["out"], np.float32)
        for t in range(T):
            ns = int(pc["nslots"][t])
            if ns > 0:
                fs = int(pc["first_seg"][t])
                oo = int(out_off[t])
                out_full[fs:fs + ns] += dev[oo:oo + ns]
    return out_full


LAST_EXEC_NS = None
